# revision 13
# baseline (speedup 1.0000x reference)
"""TRN2 Bass kernel: 2-layer bidirectional LSTM encoder (nn_BiLstmCellEncoder).

Full-input contract: kernel(**inputs) takes the unsharded inputs of
reference.setup_inputs() and returns the full [128, 200, 1024] fp32 output.

Sharding: the forward chain (layer0->layer1 fwd) and backward chain are
completely independent, so work is split direction (2) x batch quarters (4)
across the 8 NeuronCores. Every core runs the SAME SPMD program: a 2-layer
unidirectional LSTM over 32 sequences; the backward direction is realized by
feeding time-reversed inputs/mask and reversing the output on the host.

Per core, per layer the program pipelines:
  - input-projection matmuls (gx = Wih @ x + b), produced one 8-step block
    ahead, interleaved 2 gate-chunks per recurrence step to keep the PE busy
  - the sequential recurrence (64 [128x128] bf16 matmuls per step)
  - gate activations on ScalarE, cell/hidden updates on VectorE
Matmul operands are bf16 (fp32 PSUM accumulation, fp32 cell state); masked
layer outputs are stored via a per-block SBUF window -> DRAM.
"""
import sys
sys.path.insert(0, '/opt/trn_rl_repo')

import numpy as np
import ml_dtypes

import concourse.bass as bass
import concourse.mybir as mybir
from concourse import bacc
import concourse.tile as tile
from concourse import bass_utils

F32 = mybir.dt.float32
BF16 = mybir.dt.bfloat16
AF = mybir.ActivationFunctionType
OP = mybir.AluOpType

B, S, D, H = 128, 200, 512, 512
NG = 4 * H
KC = 4    # 128-row chunks over H/D (contraction)
MC = 16   # 128-row chunks over the 4H gate rows
BC = 32   # batch per core
TB = 8    # time-steps per pipeline block (must divide S)
NCORES = 8

TRACE = False          # set True (e.g. from test.py) to capture a HW profile
LAST_RESULTS = None    # BassKernelResults of the most recent run


def _build():
    assert S % TB == 0
    nblocks = S // TB
    nc = bacc.Bacc("TRN2", target_bir_lowering=False, debug=False,
                   num_devices=NCORES)

    xT = nc.dram_tensor("xT", [KC, 128, S, BC], BF16, kind="ExternalInput")
    wih = [nc.dram_tensor(f"wih{l}T", [KC, 128, NG], BF16, kind="ExternalInput")
           for l in range(2)]
    whh = [nc.dram_tensor(f"whh{l}T", [KC, 128, NG], BF16, kind="ExternalInput")
           for l in range(2)]
    bias = [nc.dram_tensor(f"bias{l}T", [128, MC], F32, kind="ExternalInput")
            for l in range(2)]
    maskrep = nc.dram_tensor("maskrep", [128, S, BC], BF16, kind="ExternalInput")
    out = nc.dram_tensor("out", [KC, 128, S, BC], F32, kind="ExternalOutput")

    with tile.TileContext(nc) as tc:
        with tc.tile_pool(name="const", bufs=1) as cpool, \
             tc.tile_pool(name="state", bufs=1) as spool, \
             tc.tile_pool(name="stage", bufs=2) as stpool, \
             tc.tile_pool(name="gx", bufs=2) as gxpool, \
             tc.tile_pool(name="win", bufs=2) as wpool, \
             tc.tile_pool(name="act", bufs=4) as apool, \
             tc.tile_pool(name="dram", bufs=1, space="DRAM") as dpool, \
             tc.tile_pool(name="psg", bufs=2, space="PSUM") as psg_pool, \
             tc.tile_pool(name="psp", bufs=2, space="PSUM") as psp_pool:

            wih_sb = [cpool.tile([128, KC, NG], BF16, tag=f"wih{l}",
                                 name=f"wih{l}sb") for l in range(2)]
            whh_sb = [cpool.tile([128, KC, NG], BF16, tag=f"whh{l}",
                                 name=f"whh{l}sb") for l in range(2)]
            bias_sb = [cpool.tile([128, MC], F32, tag=f"bias{l}",
                                  name=f"bias{l}sb") for l in range(2)]
            mask_sb = cpool.tile([128, S, BC], BF16, tag="mask")
            for l in range(2):
                nc.sync.dma_start(wih_sb[l][:],
                                  wih[l].ap().rearrange("k p c -> p k c"))
                nc.sync.dma_start(whh_sb[l][:],
                                  whh[l].ap().rearrange("k p c -> p k c"))
                nc.sync.dma_start(bias_sb[l][:], bias[l].ap())
            nc.sync.dma_start(mask_sb[:], maskrep.ap())

            h_sb = spool.tile([128, KC, BC], BF16, tag="h")
            c_sb = spool.tile([128, KC, BC], F32, tag="c")

            h0m = dpool.tile([KC, 128, S, BC], BF16, tag="h0m")

            def stage_block(l, k):
                st = stpool.tile([128, KC, TB, BC], BF16, tag="stage", name="st")
                src = xT.ap() if l == 0 else h0m[:]
                nc.sync.dma_start(
                    st[:],
                    src[:, :, k * TB:(k + 1) * TB, :].rearrange(
                        "k p t b -> p k t b"))
                return st

            def produce(l, st, gx, m):
                pp = psp_pool.tile([128, TB, BC], F32, tag="psp", name="pp")
                for kc in range(KC):
                    nc.tensor.matmul(
                        pp[:],
                        wih_sb[l][:, kc, m * 128:(m + 1) * 128],
                        st[:, kc, :, :],
                        start=(kc == 0), stop=(kc == KC - 1))
                nc.scalar.activation(gx[:, :, m, :], pp[:], AF.Identity,
                                     bias=bias_sb[l][:, m:m + 1])

            for l in range(2):
                nc.vector.memset(h_sb[:], 0.0)
                nc.vector.memset(c_sb[:], 0.0)

                st_cur = stage_block(l, 0)
                gx_cur = gxpool.tile([128, TB, MC, BC], F32, tag="gx",
                                     name="gx0")
                for m in range(MC):
                    produce(l, st_cur, gx_cur, m)

                for k in range(nblocks):
                    if k + 1 < nblocks:
                        st_next = stage_block(l, k + 1)
                        gx_next = gxpool.tile([128, TB, MC, BC], F32, tag="gx",
                                              name="gxn")
                    win = wpool.tile([128, KC, TB, BC],
                                     BF16 if l == 0 else F32, tag=f"win{l}",
                                     name="win")
                    for j in range(TB):
                        t = k * TB + j
                        # MM order [i,g][f][o] with split PSUM tiles: the
                        # i*g product (the longest gate chain) starts after 32
                        # of 64 MMs; f*c overlaps; the o-path is the tail.
                        pgig = psg_pool.tile([128, 8, BC], F32, tag="psgig",
                                             name="pgig")
                        pgf = psg_pool.tile([128, 4, BC], F32, tag="psgf",
                                            name="pgf")
                        pgo = psg_pool.tile([128, 4, BC], F32, tag="psgo",
                                            name="pgo")
                        # m-chunk order: i(0-3), g(8-11), f(4-7), o(12-15)
                        mm_order = [(0, pgig, 0), (1, pgig, 1), (2, pgig, 2),
                                    (3, pgig, 3), (8, pgig, 4), (9, pgig, 5),
                                    (10, pgig, 6), (11, pgig, 7),
                                    (4, pgf, 0), (5, pgf, 1), (6, pgf, 2),
                                    (7, pgf, 3),
                                    (12, pgo, 0), (13, pgo, 1), (14, pgo, 2),
                                    (15, pgo, 3)]
                        for m, dtile, dm in mm_order:
                            for kc in range(KC):
                                nc.tensor.matmul(
                                    dtile[:, dm, :],
                                    whh_sb[l][:, kc, m * 128:(m + 1) * 128],
                                    h_sb[:, kc, :],
                                    start=(kc == 0), stop=(kc == KC - 1))
                        if k + 1 < nblocks:
                            for mm in range(j * MC // TB,
                                            (j + 1) * MC // TB):
                                produce(l, st_next, gx_next, mm)
                        gsig = apool.tile([128, 8, BC], F32, tag="gsig",
                                          name="gsig")
                        gxi = gx_cur[:, j, 0:4, :]
                        gxg = gx_cur[:, j, 8:12, :]
                        nc.vector.tensor_tensor(gsig[:, 0:4, :], pgig[:, 0:4, :],
                                                gxi, OP.add)
                        nc.vector.tensor_tensor(gsig[:, 4:8, :], pgig[:, 4:8, :],
                                                gxg, OP.add)
                        gsf = apool.tile([128, 4, BC], F32, tag="gsf",
                                         name="gsf")
                        nc.vector.tensor_tensor(gsf[:], pgf[:],
                                                gx_cur[:, j, 4:8, :], OP.add)
                        gso = apool.tile([128, 4, BC], F32, tag="gso",
                                         name="gso")
                        nc.vector.tensor_tensor(gso[:], pgo[:],
                                                gx_cur[:, j, 12:16, :], OP.add)
                        s_i = apool.tile([128, 4, BC], F32, tag="s_i",
                                         name="s_i")
                        t_g = apool.tile([128, 4, BC], F32, tag="t_g",
                                         name="t_g")
                        s_f = apool.tile([128, 4, BC], F32, tag="s_f",
                                         name="s_f")
                        s_o = apool.tile([128, 4, BC], F32, tag="s_o",
                                         name="s_o")
                        nc.scalar.activation(s_i[:], gsig[:, 0:4, :], AF.Sigmoid)
                        nc.scalar.activation(t_g[:], gsig[:, 4:8, :], AF.Tanh)
                        nc.scalar.activation(s_f[:], gsf[:], AF.Sigmoid)
                        nc.scalar.activation(s_o[:], gso[:], AF.Sigmoid)
                        tmp = apool.tile([128, 4, BC], F32, tag="tmp",
                                         name="tmp")
                        nc.vector.tensor_tensor(tmp[:], s_i[:], t_g[:], OP.mult)
                        nc.vector.tensor_tensor(c_sb[:], c_sb[:], s_f[:],
                                                OP.mult)
                        nc.vector.tensor_tensor(c_sb[:], c_sb[:], tmp[:],
                                                OP.add)
                        tc_t = apool.tile([128, 4, BC], F32, tag="tc",
                                          name="tc_t")
                        nc.scalar.activation(tc_t[:], c_sb[:], AF.Tanh)
                        nc.vector.tensor_tensor(h_sb[:], s_o[:], tc_t[:],
                                                OP.mult)
                        nc.vector.tensor_tensor(
                            win[:, :, j, :], h_sb[:],
                            mask_sb[:, t, None, :].to_broadcast([128, KC, BC]),
                            OP.mult)
                    dst = h0m[:] if l == 0 else out.ap()
                    nc.sync.dma_start(
                        dst[:, :, k * TB:(k + 1) * TB, :].rearrange(
                            "k p t b -> p k t b"),
                        win[:])
                    if k + 1 < nblocks:
                        st_cur, gx_cur = st_next, gx_next
    nc.compile()
    return nc


_NC = None


def _get_nc():
    global _NC
    if _NC is None:
        _NC = _build()
    return _NC


def _prep_in_maps(x, lens, Wih_f, Whh_f, bih_f, bhh_f, Wih_b, Whh_b,
                  bih_b, bhh_b):
    bf = ml_dtypes.bfloat16
    x = np.asarray(x, dtype=np.float32)
    lens_np = np.asarray(lens).astype(np.int64)
    valid_full = (np.arange(S)[None, :] < lens_np[:, None]).astype(np.float32)

    Ws = {0: (np.asarray(Wih_f), np.asarray(Whh_f),
              np.asarray(bih_f), np.asarray(bhh_f)),
          1: (np.asarray(Wih_b), np.asarray(Whh_b),
              np.asarray(bih_b), np.asarray(bhh_b))}

    in_maps = []
    for c in range(NCORES):
        dirn, q = c // 4, c % 4
        bsl = slice(q * BC, (q + 1) * BC)
        xs = x[bsl]
        valid = valid_full[bsl]
        if dirn == 1:
            xs = xs[:, ::-1]
            valid = valid[:, ::-1]
        Wihs, Whhs, bihs, bhhs = Ws[dirn]
        m = {
            "xT": np.ascontiguousarray(xs.transpose(2, 1, 0)).reshape(
                KC, 128, S, BC).astype(bf),
            "maskrep": np.broadcast_to(valid.T[None], (128, S, BC)).astype(bf)
                .copy(),
        }
        for l in range(2):
            m[f"wih{l}T"] = np.ascontiguousarray(Wihs[l].T).reshape(
                KC, 128, NG).astype(bf)
            m[f"whh{l}T"] = np.ascontiguousarray(Whhs[l].T).reshape(
                KC, 128, NG).astype(bf)
            m[f"bias{l}T"] = np.ascontiguousarray(
                (bihs[l] + bhhs[l]).astype(np.float32).reshape(MC, 128).T)
        in_maps.append(m)
    return in_maps


def _assemble(results):
    outp = np.empty((B, S, 2 * H), dtype=np.float32)
    for c in range(NCORES):
        dirn, q = c // 4, c % 4
        arr = results[c]["out"].reshape(H, S, BC).transpose(2, 1, 0)
        if dirn == 1:
            arr = arr[:, ::-1, :]
        outp[q * BC:(q + 1) * BC, :, dirn * H:(dirn + 1) * H] = arr
    return outp


def kernel(x, lens, Wih_f, Whh_f, bih_f, bhh_f, Wih_b, Whh_b, bih_b, bhh_b):
    global LAST_RESULTS
    in_maps = _prep_in_maps(x, lens, Wih_f, Whh_f, bih_f, bhh_f,
                            Wih_b, Whh_b, bih_b, bhh_b)
    nc = _get_nc()
    res = bass_utils.run_bass_kernel_spmd(nc, in_maps, list(range(NCORES)),
                                          trace=TRACE)
    LAST_RESULTS = res
    return _assemble(res.results)
